# revision 65
# baseline (speedup 1.0000x reference)
"""GAT x2 + MLP heads (nn_Combined) on 8 trn2 NeuronCores — fused single launch.

Node blocks (128 rows) are assigned CONTIGUOUSLY: core c owns global blocks
[c*49, (c+1)*49).  One NEFF does: stage-A matmul on the core's own node
shard -> AllGather of the [h | a_s | a_d] 512B rows -> layer-1 edge
aggregation (dma_gather streams + one-hot mask matmuls) fused with layer-2
stage A -> second AllGather -> layer-2 aggregation -> per-graph sum-pool
partials [64, 512] via one accumulating matmul.  Host sums partials over
cores, divides by graph sizes, and runs the tiny dense heads in numpy
(<1% of FLOPs).  Softmax max-subtraction is skipped (bounded activations;
den >= exp(self-loop) > 0).

The dst block's own a_d rows are read back from the LOCAL stage-A bounce
buffer (plain dma), not gathered — removes the baseline's per-slot extra
gather chunk and keeps the SPMD program identical across cores.

A persistent JAX compilation cache makes repeat launches skip the
XLA->neuronxcc hook (which re-verifies BIR on every cache miss).
"""
import sys
sys.path.insert(0, "/opt/trn_rl_repo")
import time
import numpy as np
import jax
jax.config.update("jax_compilation_cache_dir", "/tmp/jax_cache")
jax.config.update("jax_persistent_cache_min_compile_time_secs", 0)
jax.config.update("jax_persistent_cache_min_entry_size_bytes", -1)
import concourse.bacc as bacc
import concourse.bass as bass
import concourse.mybir as mybir
import concourse.tile as tile
from concourse.masks import make_identity
from concourse.bass_utils import run_bass_kernel_spmd

F32 = mybir.dt.float32
I16 = mybir.dt.int16
I8 = mybir.dt.int8
BF16 = mybir.dt.bfloat16

N = 50000
F = 64
G = 512
H = 4
CH_ = 16
BN_EPS = 1e-5
NCORE = 8
P = 128
NBLK = (N + P - 1) // P          # 391
BPC = (NBLK + NCORE - 1) // NCORE  # 49 blocks per core
NBLKP = BPC * NCORE              # 392 (one pad block)
NPC = BPC * P                    # 6272 nodes per core
NPAD = NBLKP * P                 # 50176
NLOW = 32768                     # int16 gather-index split
NHI = NPAD - NLOW                # 17408
NG = 8                           # gather chunks (x128 idx) per dma_gather
SCRATCH = 16384


def _wrap_idx(flat):
    # compact 16-row wrap; the kernel replicates to 128 partitions on-device
    n = flat.shape[0]
    return flat.reshape(n // 16, 16).T.astype(np.int16)


def _pad8(x):
    return -(-x // 8) * 8


def _prep_graph(edge_index, batch):
    """Per-core gather streams.  Slot s (dst block c*BPC+s) owns CLP low-src
    chunks, CHP high-src chunks (each 128 edges, padded to 8-chunk gather
    groups), plus an aligned dst-row stream indexing the LOCAL stage-A
    buffer (ids < NPC, identical layout on every core)."""
    src = np.concatenate([np.asarray(edge_index[0]), np.arange(N)]).astype(np.int64)
    dst = np.concatenate([np.asarray(edge_index[1]), np.arange(N)]).astype(np.int64)
    order = np.argsort(dst, kind="stable")
    src, dst = src[order], dst[order]
    starts = np.searchsorted(dst, np.arange(0, NBLKP * P + 1, P))
    per = []
    for c in range(NCORE):
        rows = []
        for s in range(BPC):
            b = c * BPC + s
            e0, e1 = starts[b], starts[b + 1]
            es, ed = src[e0:e1], dst[e0:e1] - P * b
            m = es < NLOW
            rows.append((es[m], ed[m], es[~m] - NLOW, ed[~m]))
        per.append(rows)
    CLr = max(1, max(-(-len(r[0]) // P) for rows in per for r in rows))
    CHr = max(1, max(-(-len(r[2]) // P) for rows in per for r in rows))
    CL, CH = _pad8(CLr), _pad8(CHr)
    NCH = CL + CH
    TL, TH, TD = BPC * CL, BPC * CH, BPC * NCH
    idxL = np.zeros((NCORE, 16, TL * 8), np.int16)
    idxH = np.zeros((NCORE, 16, TH * 8), np.int16)
    idxD = np.zeros((NCORE, 16, TD * 8), np.int16)
    dl = np.full((NCORE, P, BPC * NCH), -1, np.int8)
    bl = np.full((NCORE, P, BPC), -1.0, np.float32)
    batch = np.asarray(batch).astype(np.int64)
    for c in range(NCORE):
        for s in range(BPC):
            le, ld, he, hd = per[c][s]
            fl = np.zeros(CL * P, np.int64); fl[:len(le)] = le
            dv = np.full(CL * P, -1, np.int64); dv[:len(ld)] = ld
            for j in range(CL):
                idxL[c][:, (s * CL + j) * 8:(s * CL + j + 1) * 8] = \
                    _wrap_idx(fl[j * P:(j + 1) * P].astype(np.int16))
            dl[c, :, s * NCH:s * NCH + CL] = dv.reshape(CL, P).T
            fh = np.zeros(CH * P, np.int64); fh[:len(he)] = he
            dvh = np.full(CH * P, -1, np.int64); dvh[:len(hd)] = hd
            for j in range(CH):
                idxH[c][:, (s * CH + j) * 8:(s * CH + j + 1) * 8] = \
                    _wrap_idx(fh[j * P:(j + 1) * P].astype(np.int16))
            dl[c, :, s * NCH + CL:(s + 1) * NCH] = dvh.reshape(CH, P).T
            # dst-row stream: local row = s*128 + dst_offset, aligned 1:1
            # with the low|high chunk positions of this slot
            dall = np.concatenate([dv, dvh])
            didx = np.where(dall >= 0, s * P + dall, 0)
            for j in range(NCH):
                idxD[c][:, (s * NCH + j) * 8:(s * NCH + j + 1) * 8] = \
                    _wrap_idx(didx[j * P:(j + 1) * P].astype(np.int16))
            n0 = P * (c * BPC + s)
            rows = max(0, min(P, N - n0))
            if rows > 0:
                bl[c, :rows, s] = batch[n0:n0 + rows].astype(np.float32)
    return CL, CH, CLr, CHr, idxL, idxH, idxD, dl, bl


def _blob_layout(CL, CH):
    """(name, nbytes) sections of the per-core packed input blob."""
    NCH = CL + CH
    TL, TH, TD = BPC * CL, BPC * CH, BPC * NCH
    return [
        ("xT", F * NPC),                # fp8 e4m3 [F, NPC]
        ("idxL", 16 * TL * 8 * 2),      # i16 [16, TL*8]
        ("idxH", 16 * TH * 8 * 2),      # i16 [16, TH*8]
        ("idxD", 16 * TD * 8 * 2),      # i16 [16, TD*8]
        ("dl", P * BPC * NCH),          # i8  [P, BPC*NCH]
        ("bl", P * BPC * 4),            # f32 [P, BPC]
        ("wc1", F * 72 * 2),            # bf16 [F, 72]
        ("wc2", F * 72 * 2),
        ("cst1", 3 * P * F * 2),        # bf16 [3, P, F]
        ("cst2", 3 * P * F * 2),
    ]


def _blob_offsets(CL, CH):
    offs, o = {}, 0
    for name, nb in _blob_layout(CL, CH):
        offs[name] = (o, nb)
        o += nb
    return offs, o


def _build_fused(CL, CH, CLr=None, CHr=None, probe=None):
    NCH = CL + CH
    TL, TH, TD = BPC * CL, BPC * CH, BPC * NCH
    NB = NCH // 8                    # 8-chunk batches per slot
    CLr = CL if CLr is None else CLr
    CHr = CH if CHr is None else CHr
    # chunk positions that are padding on EVERY core (skip their matmuls)
    dead = set(range(CLr, CL)) | set(range(CL + CHr, NCH))
    offs, TOT = _blob_offsets(CL, CH)
    nc = bacc.Bacc("TRN2", target_bir_lowering=False, debug=False,
                   dynamic_dma_scratch_size=SCRATCH)
    blob = nc.dram_tensor("blob", [TOT], I8, kind="ExternalInput")
    pooledT = nc.dram_tensor("pooledT", [F, G], BF16, kind="ExternalOutput")

    def sec(name, dt, p, sub=0, subsz=None):
        o, nb = offs[name]
        if subsz is not None:
            o, nb = o + sub * subsz, subsz
        return blob[o:o + nb].bitcast(dt).rearrange("(p c) -> p c", p=p)
    sa1_in = nc.dram_tensor("sa1_in", [NPC, P], BF16)
    sa1 = nc.dram_tensor("sa1", [NPAD, P], BF16, addr_space="Shared")
    sa2_in = nc.dram_tensor("sa2_in", [NPC, P], BF16)
    sa2 = nc.dram_tensor("sa2", [NPAD, P], BF16, addr_space="Shared")
    A = mybir.ActivationFunctionType
    RG = [list(range(NCORE))]

    with tile.TileContext(nc) as tc:
        with tc.tile_pool(name="const", bufs=1) as cp:
            ident = cp.tile([P, P], F32)
            make_identity(nc, ident[:])
            iot32 = cp.tile([P, 512], mybir.dt.int32)
            nc.gpsimd.iota(iot32[:], pattern=[[1, 512]], channel_multiplier=0)
            iota5 = cp.tile([P, 512], F32)
            nc.vector.tensor_copy(out=iota5[:], in_=iot32[:])
            # 0..127 repeated NCH x along the free dim (slot-wide one-hots)
            iotar = cp.tile([P, NCH * P], BF16)
            for g in range(NCH):
                nc.vector.tensor_copy(out=iotar[:, g * P:(g + 1) * P],
                                      in_=iota5[:, 0:P])
            wct1 = cp.tile([F, 72], BF16)
            nc.sync.dma_start(wct1[:], sec("wc1", BF16, F))
            wct2 = cp.tile([F, 72], BF16)
            nc.sync.dma_start(wct2[:], sec("wc2", BF16, F))
            PF2 = P * F * 2
            cstt = cp.tile([P, 6 * F], BF16)
            for i in range(3):
                nc.sync.dma_start(cstt[:, i * F:(i + 1) * F],
                                  sec("cst1", BF16, P, i, PF2))
                nc.sync.dma_start(cstt[:, (3 + i) * F:(4 + i) * F],
                                  sec("cst2", BF16, P, i, PF2))
            cstf = cp.tile([P, 6 * F], F32)
            nc.vector.tensor_copy(out=cstf[:], in_=cstt[:])
            gbt1, sst1, tst1 = (cstf[:, 0:F], cstf[:, F:2 * F], cstf[:, 2 * F:3 * F])
            gbt2, sst2, tst2 = (cstf[:, 3 * F:4 * F], cstf[:, 4 * F:5 * F],
                                cstf[:, 5 * F:6 * F])
            ilt = cp.tile([P, TL * 8], I16)
            iht = cp.tile([P, TH * 8], I16)
            idt = cp.tile([P, TD * 8], I16)
            for k in range(8):
                nc.sync.dma_start(ilt[16 * k:16 * (k + 1), :], sec("idxL", I16, 16))
                nc.sync.dma_start(iht[16 * k:16 * (k + 1), :], sec("idxH", I16, 16))
                nc.sync.dma_start(idt[16 * k:16 * (k + 1), :], sec("idxD", I16, 16))
            dlt8 = cp.tile([P, BPC * NCH], I8)
            nc.sync.dma_start(dlt8[:], sec("dl", I8, P))
            dlt = cp.tile([P, BPC * NCH], BF16)
            nc.vector.tensor_copy(out=dlt[:], in_=dlt8[:])
            blt = cp.tile([P, BPC], F32)
            nc.sync.dma_start(blt[:], sec("bl", F32, P))
            xt8 = cp.tile([F, NPC], mybir.dt.float8e4)
            nc.sync.dma_start(xt8[:], sec("xT", mybir.dt.float8e4, F))
            xtb = cp.tile([F, NPC], BF16)
            nc.vector.tensor_copy(out=xtb[:], in_=xt8[:])

            # ---- stage A, layer 1 (own shard only) ----
            with (tc.tile_pool(name="sap", bufs=2, space="PSUM") as sap,
                  tc.tile_pool(name="sas", bufs=3) as sas):
                for b in range(BPC):
                    ps = sap.tile([P, 72], F32, tag="ps")
                    nc.tensor.matmul(out=ps[:], lhsT=xtb[:, P * b:P * (b + 1)],
                                     rhs=wct1[:], start=True, stop=True)
                    st = sas.tile([P, P], BF16, tag="st")
                    nc.scalar.activation(out=st[:, :72], in_=ps[:], func=A.Copy)
                    nc.vector.memset(st[:, 72:], 0.0)
                    nc.sync.dma_start(sa1_in[P * b:P * (b + 1), :], st[:])

            nc.gpsimd.collective_compute(
                "AllGather", mybir.AluOpType.bypass, replica_groups=RG,
                ins=[sa1_in[:].opt()], outs=[sa1[:].opt()])

            def aggregate(sa_full, sa_loc, gb, ss, ts, epilogue):
                """One GAT edge-aggregation pass over the core's BPC blocks.
                Slot s = NB gather-group batches of 8 chunks (128 edges each);
                per batch the mask/softmax/message ops run 8 chunks wide."""
                src_aps = {"l": sa_full[0:NLOW, :], "h": sa_full[NLOW:NPAD, :],
                           "d": sa_loc[:]}
                idx_tiles = {"l": ilt, "h": iht, "d": idt}
                with (tc.tile_pool(name="gat", bufs=3) as gp,
                      tc.tile_pool(name="mk", bufs=3) as mk,
                      tc.tile_pool(name="sm", bufs=3) as sm,
                      tc.tile_pool(name="ep", bufs=2) as epp,
                      tc.tile_pool(name="pst", bufs=2, space="PSUM") as pst,
                      tc.tile_pool(name="psa", bufs=2, space="PSUM") as psa,
                      tc.tile_pool(name="pso", bufs=2, space="PSUM") as pso):
                    cache = {}

                    def group(stream, t):
                        """[P, 8, 128] view of gather-group t of a stream."""
                        key = (stream, t)
                        if key not in cache:
                            gt = gp.tile([P, NG * P], BF16, tag="g" + stream,
                                         name="gt_" + stream)
                            nc.gpsimd.dma_gather(
                                out_ap=gt[:].rearrange("p (c e) -> p c e", e=P),
                                in_ap=src_aps[stream],
                                idxs_ap=idx_tiles[stream][:, t * 64:(t + 1) * 64],
                                num_idxs=NG * P, num_idxs_reg=NG * P, elem_size=P)
                            cache[key] = gt
                        return cache[key][:].rearrange("p (c e) -> p c e", e=P)

                    # Q-tiled BN constants for the batched epilogue
                    Q = 7
                    bnq = mk.tile([P, 3 * Q * F], F32, tag="bnq", bufs=1,
                                  name="bnq")
                    for q in range(Q):
                        nc.vector.tensor_copy(out=bnq[:, q * F:(q + 1) * F], in_=gb)
                        nc.vector.tensor_copy(
                            out=bnq[:, (Q + q) * F:(Q + q + 1) * F], in_=ss)
                        nc.vector.tensor_copy(
                            out=bnq[:, (2 * Q + q) * F:(2 * Q + q + 1) * F], in_=ts)

                    nslot = BPC // 2 if probe == "half" else BPC
                    stg = None
                    batch_s0 = 0
                    for s in range(nslot):
                        if s % Q == 0:
                            stg = sm.tile([P, Q * 68], F32, tag="stg", bufs=2,
                                          name="stg")
                            batch_s0 = s
                        acc = psa.tile([P, 68], F32, tag="acc", name="acc")
                        gts = []
                        for b in range(NB):
                            gts.append((group("l" if b < CL // 8 else "h",
                                              s * (CL // 8) + b if b < CL // 8
                                              else s * (CH // 8) + (b - CL // 8)),
                                        group("d", s * NB + b)))
                        # slot-wide one-hot mask and logits
                        S8 = mk.tile([P, NCH * P], BF16, tag="S8", name="S8")
                        nc.vector.tensor_tensor(
                            out=S8[:].rearrange("p (c e) -> p c e", e=P),
                            in0=iotar[:].rearrange("p (c e) -> p c e", e=P),
                            in1=dlt[:, s * NCH:(s + 1) * NCH].to_broadcast(
                                [P, NCH, P]),
                            op=mybir.AluOpType.is_equal)
                        e1 = sm.tile([P, NCH * 4], BF16, tag="e1", name="e1")
                        e13 = e1[:].rearrange("p (c e) -> p c e", e=4)
                        for b, (g3s, g3d) in enumerate(gts):
                            nc.vector.tensor_tensor(
                                out=e13[:, b * 8:(b + 1) * 8, :],
                                in0=g3s[:, :, 64:68], in1=g3d[:, :, 68:72],
                                op=mybir.AluOpType.add)
                        ex = sm.tile([P, NCH * 4], BF16, tag="ex", name="ex")
                        nc.scalar.activation(out=ex[:], in_=e1[:], func=A.Lrelu,
                                             alpha=0.2)
                        nc.scalar.activation(out=ex[:], in_=ex[:], func=A.Exp)
                        msg = sm.tile([P, NCH * 68], BF16, tag="msg", name="msg")
                        msg3 = msg[:].rearrange("p (c e) -> p c e", e=68)
                        nc.vector.tensor_copy(out=msg3[:, :, 64:68],
                                              in_=ex[:].rearrange(
                                                  "p (c e) -> p c e", e=4))
                        for b, (g3s, g3d) in enumerate(gts):
                            nc.vector.tensor_tensor(
                                out=msg3[:, b * 8:(b + 1) * 8, 0:64],
                                in0=g3s[:, :, 0:64],
                                in1=ex[:, b * 32:(b + 1) * 32].to_broadcast(
                                    [P, 32, 16]),
                                op=mybir.AluOpType.mult)
                        for j in range(NCH):
                            if j in dead:
                                continue
                            nc.tensor.matmul(
                                out=acc[:], lhsT=S8[:, j * P:(j + 1) * P],
                                rhs=msg[:, j * 68:(j + 1) * 68],
                                start=(j == 0), stop=(j == CL + CHr - 1))
                        qi = s - batch_s0
                        # stage to SBUF; +1e-16 keeps pad-row den > 0 and is
                        # a no-op on real feature/den magnitudes
                        nc.scalar.activation(out=stg[:, qi * 68:(qi + 1) * 68],
                                             in_=acc[:], func=A.Copy, bias=1e-16)
                        if qi == Q - 1 or s == nslot - 1:
                            nq = qi + 1
                            stg3 = stg[:].rearrange("p (c e) -> p c e", e=68)
                            rdq = epp.tile([P, Q * 4], F32, tag="rdq", name="rdq")
                            nc.vector.reciprocal(
                                rdq[:, :nq * 4].rearrange("p (c e) -> p c e", e=4),
                                stg3[:, 0:nq, 64:68])
                            hgq = epp.tile([P, Q * F], F32, tag="hgq", name="hgq")
                            nc.vector.tensor_tensor(
                                out=hgq[:, :nq * F].rearrange("p (c e) -> p c e",
                                                              e=F),
                                in0=stg3[:, 0:nq, 0:64],
                                in1=rdq[:, :nq * 4].to_broadcast([P, nq * 4, 16]),
                                op=mybir.AluOpType.mult)
                            nc.vector.tensor_tensor(out=hgq[:, :nq * F],
                                                    in0=hgq[:, :nq * F],
                                                    in1=bnq[:, :nq * F],
                                                    op=mybir.AluOpType.add)
                            nc.vector.tensor_scalar_max(hgq[:, :nq * F],
                                                        hgq[:, :nq * F], 0.0)
                            nc.vector.tensor_tensor(
                                out=hgq[:, :nq * F], in0=hgq[:, :nq * F],
                                in1=bnq[:, Q * F:Q * F + nq * F],
                                op=mybir.AluOpType.mult)
                            nc.vector.tensor_tensor(
                                out=hgq[:, :nq * F], in0=hgq[:, :nq * F],
                                in1=bnq[:, 2 * Q * F:2 * Q * F + nq * F],
                                op=mybir.AluOpType.add)
                            for q in range(nq):
                                epilogue(batch_s0 + q,
                                         hgq[:, q * F:(q + 1) * F],
                                         mk, sm, epp, pst, pso)

            # ---- layer-1 aggregation, fused with layer-2 stage A ----
            def epi1(s, hg, mk, sm, epp, pst, pso):
                hgT_p = pst.tile([F, P], F32, tag="tp")
                nc.tensor.transpose(out=hgT_p[:], in_=hg, identity=ident[:])
                hgT = epp.tile([F, P], BF16, tag="hgT")
                nc.scalar.activation(out=hgT[:], in_=hgT_p[:], func=A.Copy)
                ps2 = pso.tile([P, 72], F32, tag="ps2")
                nc.tensor.matmul(out=ps2[:], lhsT=hgT[:], rhs=wct2[:],
                                 start=True, stop=True)
                st2 = sm.tile([P, P], BF16, tag="st2")
                nc.scalar.activation(out=st2[:, :72], in_=ps2[:], func=A.Copy)
                nc.vector.memset(st2[:, 72:], 0.0)
                nc.sync.dma_start(sa2_in[P * s:P * (s + 1), :], st2[:])

            aggregate(sa1, sa1_in, gbt1, sst1, tst1, epi1)

            nc.gpsimd.collective_compute(
                "AllGather", mybir.AluOpType.bypass, replica_groups=RG,
                ins=[sa2_in[:].opt()], outs=[sa2[:].opt()])

            # ---- layer-2 aggregation, fused with sum-pool partials ----
            pooled_holder = {}

            SLAST = (BPC // 2 if probe == "half" else BPC) - 1

            def epi2(s, hg, mk, sm, epp, pst, pso):
                if "ps" not in pooled_holder:
                    pooled_holder["ps"] = pso.tile([F, 512], F32, tag="pool",
                                                   bufs=1, name="pooled_ps")
                pm = mk.tile([P, 512], F32, tag="pm")
                nc.vector.tensor_scalar(
                    out=pm[:], in0=iota5[:], scalar1=blt[:, s:s + 1],
                    scalar2=None, op0=mybir.AluOpType.is_equal)
                nc.tensor.matmul(out=pooled_holder["ps"][:], lhsT=hg, rhs=pm[:],
                                 start=(s == 0), stop=(s == SLAST))
                if s == SLAST:
                    po = epp.tile([F, 512], BF16, tag="po")
                    nc.scalar.activation(out=po[:], in_=pooled_holder["ps"][:],
                                         func=A.Copy)
                    nc.sync.dma_start(pooledT[:], po[:])

            aggregate(sa2, sa2_in, gbt2, sst2, tst2, epi2)
    nc.compile()
    # The PJRT lowering re-serializes the BIR module (to_json_bytes) on
    # every launch; the module is frozen after compile, so memoize it.
    _json = nc.to_json_bytes()
    nc.to_json_bytes = lambda: _json
    return nc


def _fold_bn(g, b, m, v):
    s = np.asarray(g) / np.sqrt(np.asarray(v) + BN_EPS)
    return s.astype(np.float32), (np.asarray(b) - np.asarray(m) * s).astype(np.float32)


def _layer_consts(W, bias, asrc, adst, bn_g, bn_b, bn_m, bn_v):
    W = np.asarray(W, np.float32)
    As = np.zeros((F, H), np.float32)
    Ad = np.zeros((F, H), np.float32)
    for hd in range(H):
        As[hd * CH_:(hd + 1) * CH_, hd] = np.asarray(asrc)[hd]
        Ad[hd * CH_:(hd + 1) * CH_, hd] = np.asarray(adst)[hd]
    wcm = np.concatenate([W, W @ As, W @ Ad], axis=1).astype(np.float32)
    s, t = _fold_bn(bn_g, bn_b, bn_m, bn_v)
    cst = np.stack([
        np.tile(np.asarray(bias, np.float32)[None, :], (P, 1)),
        np.tile(s[None, :], (P, 1)),
        np.tile(t[None, :], (P, 1)),
    ]).astype(np.float32)
    return wcm, cst


def _sigmoid(x):
    return 1.0 / (1.0 + np.exp(-x))


def _bn_np(x, g, b, m, v):
    return (x - m) / np.sqrt(v + BN_EPS) * g + b


def _heads(inp, pooled):
    f = lambda k: np.asarray(inp[k], np.float32)
    ya = np.maximum(pooled @ f("la1_w") + f("la1_b"), 0.0)
    xa = _sigmoid(ya @ f("la2_w") + f("la2_b"))            # [G, 1]
    z = f("x2")
    for i in (1, 2, 3):
        z = np.maximum(_bn_np(z @ f(f"lb{i}_w") + f(f"lb{i}_b"),
                              f(f"bnb{i}_g"), f(f"bnb{i}_b"),
                              f(f"bnb{i}_m"), f(f"bnb{i}_v")), 0.0)
    xb = _sigmoid(z @ f("lb4_w") + f("lb4_b"))             # [G, 64]
    c = np.concatenate([xa, xb], axis=1)                   # [G, 65]
    yc = np.maximum(c @ f("lc1_w") + f("lc1_b"), 0.0)
    return _sigmoid(yc @ f("lc2_w") + f("lc2_b")).astype(np.float32)


_CACHE = {}
LAUNCH_S = []      # all launches ever (name, wall seconds)
LAST_CALL = []     # launches of the most recent kernel() call


def kernel(**inputs):
    global LAST_CALL
    edge_index = inputs["edge_index"]
    batch = np.asarray(inputs["batch"]).astype(np.int64)
    CL, CH, CLr, CHr, idxL, idxH, idxD, dl, bl = _prep_graph(edge_index, batch)

    key = (CL, CH, CLr, CHr)
    if key not in _CACHE:
        _CACHE[key] = _build_fused(CL, CH, CLr, CHr)
    nc = _CACHE[key]

    w1c, cst1 = _layer_consts(inputs["gW1"], inputs["gb1"], inputs["asrc1"],
                              inputs["adst1"], inputs["bn1_g"], inputs["bn1_b"],
                              inputs["bn1_m"], inputs["bn1_v"])
    w2c, cst2 = _layer_consts(inputs["gW2"], inputs["gb2"], inputs["asrc2"],
                              inputs["adst2"], inputs["bn2_g"], inputs["bn2_b"],
                              inputs["bn2_m"], inputs["bn2_v"])
    import ml_dtypes
    x1T = np.zeros((F, NPAD), ml_dtypes.float8_e4m3)
    x1T[:, :N] = np.asarray(inputs["x1"], np.float32).T.astype(ml_dtypes.float8_e4m3)

    def pack(c):
        parts = [np.ascontiguousarray(x1T[:, c * NPC:(c + 1) * NPC]),
                 idxL[c], idxH[c], idxD[c], dl[c], bl[c],
                 w1c.astype(ml_dtypes.bfloat16), w2c.astype(ml_dtypes.bfloat16),
                 cst1.astype(ml_dtypes.bfloat16), cst2.astype(ml_dtypes.bfloat16)]
        return np.concatenate([p.reshape(-1).view(np.int8) for p in parts])

    maps = [{"blob": pack(c)} for c in range(NCORE)]
    t0 = time.time()
    res = run_bass_kernel_spmd(nc, maps, core_ids=list(range(NCORE)))
    dt = time.time() - t0
    LAUNCH_S.append(("FUSED", dt))
    LAST_CALL = [("FUSED", dt)]

    poolT = np.zeros((F, G), np.float32)
    for c in range(NCORE):
        poolT += res.results[c]["pooledT"].astype(np.float32)
    cnt = np.bincount(batch, minlength=G).astype(np.float32)
    pooled = (poolT / np.maximum(cnt, 1.0)[None, :]).T     # [G, F]
    return _heads(inputs, pooled)


# revision 66
# speedup vs baseline: 1.1750x; 1.1750x over previous
"""GAT x2 + MLP heads (nn_Combined) on 8 trn2 NeuronCores — fused single launch.

Node blocks (128 rows) are assigned CONTIGUOUSLY: core c owns global blocks
[c*49, (c+1)*49).  One NEFF does: stage-A matmul on the core's own node
shard -> AllGather of the [h | a_s | a_d] 512B rows -> layer-1 edge
aggregation (dma_gather streams + one-hot mask matmuls) fused with layer-2
stage A -> second AllGather -> layer-2 aggregation -> per-graph sum-pool
partials [64, 512] via one accumulating matmul.  Host sums partials over
cores, divides by graph sizes, and runs the tiny dense heads in numpy
(<1% of FLOPs).  Softmax max-subtraction is skipped (bounded activations;
den >= exp(self-loop) > 0).

The dst block's own a_d rows are read back from the LOCAL stage-A bounce
buffer (plain dma), not gathered — removes the baseline's per-slot extra
gather chunk and keeps the SPMD program identical across cores.

A persistent JAX compilation cache makes repeat launches skip the
XLA->neuronxcc hook (which re-verifies BIR on every cache miss).
"""
import sys
sys.path.insert(0, "/opt/trn_rl_repo")
import time
import numpy as np
import jax
jax.config.update("jax_compilation_cache_dir", "/tmp/jax_cache")
jax.config.update("jax_persistent_cache_min_compile_time_secs", 0)
jax.config.update("jax_persistent_cache_min_entry_size_bytes", -1)
import concourse.bacc as bacc
import concourse.bass as bass
import concourse.mybir as mybir
import concourse.tile as tile
from concourse.masks import make_identity
from concourse.bass_utils import run_bass_kernel_spmd

F32 = mybir.dt.float32
I16 = mybir.dt.int16
I8 = mybir.dt.int8
BF16 = mybir.dt.bfloat16

N = 50000
F = 64
G = 512
H = 4
CH_ = 16
BN_EPS = 1e-5
NCORE = 8
P = 128
NBLK = (N + P - 1) // P          # 391
BPC = (NBLK + NCORE - 1) // NCORE  # 49 blocks per core
NBLKP = BPC * NCORE              # 392 (one pad block)
NPC = BPC * P                    # 6272 nodes per core
NPAD = NBLKP * P                 # 50176
NLOW = 32768                     # int16 gather-index split
NHI = NPAD - NLOW                # 17408
NG = 8                           # gather chunks (x128 idx) per dma_gather
SCRATCH = 16384


def _wrap_idx(flat):
    # compact 16-row wrap; the kernel replicates to 128 partitions on-device
    n = flat.shape[0]
    return flat.reshape(n // 16, 16).T.astype(np.int16)


def _pad8(x):
    return -(-x // 8) * 8


def _prep_graph(edge_index, batch):
    """Per-core gather streams.  Slot s (dst block c*BPC+s) owns CLP low-src
    chunks, CHP high-src chunks (each 128 edges, padded to 8-chunk gather
    groups), plus an aligned dst-row stream indexing the LOCAL stage-A
    buffer (ids < NPC, identical layout on every core)."""
    src = np.concatenate([np.asarray(edge_index[0]), np.arange(N)]).astype(np.int64)
    dst = np.concatenate([np.asarray(edge_index[1]), np.arange(N)]).astype(np.int64)
    order = np.argsort(dst, kind="stable")
    src, dst = src[order], dst[order]
    starts = np.searchsorted(dst, np.arange(0, NBLKP * P + 1, P))
    per = []
    for c in range(NCORE):
        rows = []
        for s in range(BPC):
            b = c * BPC + s
            e0, e1 = starts[b], starts[b + 1]
            es, ed = src[e0:e1], dst[e0:e1] - P * b
            m = es < NLOW
            rows.append((es[m], ed[m], es[~m] - NLOW, ed[~m]))
        per.append(rows)
    CLr = max(1, max(-(-len(r[0]) // P) for rows in per for r in rows))
    CHr = max(1, max(-(-len(r[2]) // P) for rows in per for r in rows))
    CL, CH = _pad8(CLr), _pad8(CHr)
    NCH = CL + CH
    TL, TH, TD = BPC * CL, BPC * CH, BPC * NCH
    idxL = np.zeros((NCORE, 16, TL * 8), np.int16)
    idxH = np.zeros((NCORE, 16, TH * 8), np.int16)
    idxD = np.zeros((NCORE, 16, TD * 8), np.int16)
    dl = np.full((NCORE, P, BPC * NCH), -1, np.int8)
    bl = np.full((NCORE, P, BPC), -1.0, np.float32)
    batch = np.asarray(batch).astype(np.int64)
    for c in range(NCORE):
        for s in range(BPC):
            le, ld, he, hd = per[c][s]
            fl = np.zeros(CL * P, np.int64); fl[:len(le)] = le
            dv = np.full(CL * P, -1, np.int64); dv[:len(ld)] = ld
            for j in range(CL):
                idxL[c][:, (s * CL + j) * 8:(s * CL + j + 1) * 8] = \
                    _wrap_idx(fl[j * P:(j + 1) * P].astype(np.int16))
            dl[c, :, s * NCH:s * NCH + CL] = dv.reshape(CL, P).T
            fh = np.zeros(CH * P, np.int64); fh[:len(he)] = he
            dvh = np.full(CH * P, -1, np.int64); dvh[:len(hd)] = hd
            for j in range(CH):
                idxH[c][:, (s * CH + j) * 8:(s * CH + j + 1) * 8] = \
                    _wrap_idx(fh[j * P:(j + 1) * P].astype(np.int16))
            dl[c, :, s * NCH + CL:(s + 1) * NCH] = dvh.reshape(CH, P).T
            # dst-row stream: local row = s*128 + dst_offset, aligned 1:1
            # with the low|high chunk positions of this slot
            dall = np.concatenate([dv, dvh])
            didx = np.where(dall >= 0, s * P + dall, 0)
            for j in range(NCH):
                idxD[c][:, (s * NCH + j) * 8:(s * NCH + j + 1) * 8] = \
                    _wrap_idx(didx[j * P:(j + 1) * P].astype(np.int16))
            n0 = P * (c * BPC + s)
            rows = max(0, min(P, N - n0))
            if rows > 0:
                bl[c, :rows, s] = batch[n0:n0 + rows].astype(np.float32)
    return CL, CH, CLr, CHr, idxL, idxH, idxD, dl, bl


def _blob_layout(CL, CH):
    """(name, nbytes) sections of the per-core packed input blob."""
    NCH = CL + CH
    TL, TH, TD = BPC * CL, BPC * CH, BPC * NCH
    return [
        ("xT", F * NPC),                # fp8 e4m3 [F, NPC]
        ("idxL", 16 * TL * 8 * 2),      # i16 [16, TL*8]
        ("idxH", 16 * TH * 8 * 2),      # i16 [16, TH*8]
        ("idxD", 16 * TD * 8 * 2),      # i16 [16, TD*8]
        ("dl", P * BPC * NCH),          # i8  [P, BPC*NCH]
        ("bl", P * BPC * 4),            # f32 [P, BPC]
        ("wc1", F * 72 * 2),            # bf16 [F, 72]
        ("wc2", F * 72 * 2),
        ("cst1", 3 * P * F * 2),        # bf16 [3, P, F]
        ("cst2", 3 * P * F * 2),
    ]


def _blob_offsets(CL, CH):
    offs, o = {}, 0
    for name, nb in _blob_layout(CL, CH):
        offs[name] = (o, nb)
        o += nb
    return offs, o


def _build_fused(CL, CH, CLr=None, CHr=None, probe=None):
    NCH = CL + CH
    TL, TH, TD = BPC * CL, BPC * CH, BPC * NCH
    NB = NCH // 8                    # 8-chunk batches per slot
    CLr = CL if CLr is None else CLr
    CHr = CH if CHr is None else CHr
    # chunk positions that are padding on EVERY core (skip their matmuls)
    dead = set(range(CLr, CL)) | set(range(CL + CHr, NCH))
    offs, TOT = _blob_offsets(CL, CH)
    nc = bacc.Bacc("TRN2", target_bir_lowering=False, debug=False,
                   dynamic_dma_scratch_size=SCRATCH)
    blob = nc.dram_tensor("blob", [TOT], I8, kind="ExternalInput")
    pooledT = nc.dram_tensor("pooledT", [F, G], BF16, kind="ExternalOutput")

    def sec(name, dt, p, sub=0, subsz=None):
        o, nb = offs[name]
        if subsz is not None:
            o, nb = o + sub * subsz, subsz
        return blob[o:o + nb].bitcast(dt).rearrange("(p c) -> p c", p=p)
    sa1_in = nc.dram_tensor("sa1_in", [NPC, P], BF16)
    sa1 = nc.dram_tensor("sa1", [NPAD, P], BF16, addr_space="Shared")
    sa2_in = nc.dram_tensor("sa2_in", [NPC, P], BF16)
    sa2 = nc.dram_tensor("sa2", [NPAD, P], BF16, addr_space="Shared")
    A = mybir.ActivationFunctionType
    RG = [list(range(NCORE))]

    with tile.TileContext(nc) as tc:
        with tc.tile_pool(name="const", bufs=1) as cp:
            ident = cp.tile([P, P], F32)
            make_identity(nc, ident[:])
            iot32 = cp.tile([P, 512], mybir.dt.int32)
            nc.gpsimd.iota(iot32[:], pattern=[[1, 512]], channel_multiplier=0)
            iota5 = cp.tile([P, 512], F32)
            nc.vector.tensor_copy(out=iota5[:], in_=iot32[:])
            # 0..127 repeated 8x along the free dim (batched dst one-hots)
            iotar = cp.tile([P, 8 * P], BF16)
            for g in range(8):
                nc.vector.tensor_copy(out=iotar[:, g * P:(g + 1) * P],
                                      in_=iota5[:, 0:P])
            wct1 = cp.tile([F, 72], BF16)
            nc.sync.dma_start(wct1[:], sec("wc1", BF16, F))
            wct2 = cp.tile([F, 72], BF16)
            nc.sync.dma_start(wct2[:], sec("wc2", BF16, F))
            PF2 = P * F * 2
            cstt = cp.tile([P, 6 * F], BF16)
            for i in range(3):
                nc.sync.dma_start(cstt[:, i * F:(i + 1) * F],
                                  sec("cst1", BF16, P, i, PF2))
                nc.sync.dma_start(cstt[:, (3 + i) * F:(4 + i) * F],
                                  sec("cst2", BF16, P, i, PF2))
            cstf = cp.tile([P, 6 * F], F32)
            nc.vector.tensor_copy(out=cstf[:], in_=cstt[:])
            gbt1, sst1, tst1 = (cstf[:, 0:F], cstf[:, F:2 * F], cstf[:, 2 * F:3 * F])
            gbt2, sst2, tst2 = (cstf[:, 3 * F:4 * F], cstf[:, 4 * F:5 * F],
                                cstf[:, 5 * F:6 * F])
            ilt = cp.tile([P, TL * 8], I16)
            iht = cp.tile([P, TH * 8], I16)
            idt = cp.tile([P, TD * 8], I16)
            for k in range(8):
                nc.sync.dma_start(ilt[16 * k:16 * (k + 1), :], sec("idxL", I16, 16))
                nc.sync.dma_start(iht[16 * k:16 * (k + 1), :], sec("idxH", I16, 16))
                nc.sync.dma_start(idt[16 * k:16 * (k + 1), :], sec("idxD", I16, 16))
            dlt8 = cp.tile([P, BPC * NCH], I8)
            nc.sync.dma_start(dlt8[:], sec("dl", I8, P))
            dlt = cp.tile([P, BPC * NCH], BF16)
            nc.vector.tensor_copy(out=dlt[:], in_=dlt8[:])
            blt = cp.tile([P, BPC], F32)
            nc.sync.dma_start(blt[:], sec("bl", F32, P))
            xt8 = cp.tile([F, NPC], mybir.dt.float8e4)
            nc.sync.dma_start(xt8[:], sec("xT", mybir.dt.float8e4, F))
            xtb = cp.tile([F, NPC], BF16)
            nc.vector.tensor_copy(out=xtb[:], in_=xt8[:])

            # ---- stage A, layer 1 (own shard only) ----
            with (tc.tile_pool(name="sap", bufs=2, space="PSUM") as sap,
                  tc.tile_pool(name="sas", bufs=3) as sas):
                for b in range(BPC):
                    ps = sap.tile([P, 72], F32, tag="ps")
                    nc.tensor.matmul(out=ps[:], lhsT=xtb[:, P * b:P * (b + 1)],
                                     rhs=wct1[:], start=True, stop=True)
                    st = sas.tile([P, P], BF16, tag="st")
                    nc.scalar.activation(out=st[:, :72], in_=ps[:], func=A.Copy)
                    nc.vector.memset(st[:, 72:], 0.0)
                    nc.sync.dma_start(sa1_in[P * b:P * (b + 1), :], st[:])

            nc.gpsimd.collective_compute(
                "AllGather", mybir.AluOpType.bypass, replica_groups=RG,
                ins=[sa1_in[:].opt()], outs=[sa1[:].opt()])

            def aggregate(sa_full, sa_loc, gb, ss, ts, epilogue):
                """One GAT edge-aggregation pass over the core's BPC blocks.
                Slot s = NB gather-group batches of 8 chunks (128 edges each);
                per batch the mask/softmax/message ops run 8 chunks wide."""
                src_aps = {"l": sa_full[0:NLOW, :], "h": sa_full[NLOW:NPAD, :],
                           "d": sa_loc[:]}
                idx_tiles = {"l": ilt, "h": iht, "d": idt}
                with (tc.tile_pool(name="gat", bufs=3) as gp,
                      tc.tile_pool(name="mk", bufs=3) as mk,
                      tc.tile_pool(name="sm", bufs=3) as sm,
                      tc.tile_pool(name="ep", bufs=2) as epp,
                      tc.tile_pool(name="pst", bufs=2, space="PSUM") as pst,
                      tc.tile_pool(name="psa", bufs=2, space="PSUM") as psa,
                      tc.tile_pool(name="pso", bufs=2, space="PSUM") as pso):
                    cache = {}

                    def group(stream, t):
                        """[P, 8, 128] view of gather-group t of a stream."""
                        key = (stream, t)
                        if key not in cache:
                            gt = gp.tile([P, NG * P], BF16, tag="g" + stream,
                                         name="gt_" + stream)
                            nc.gpsimd.dma_gather(
                                out_ap=gt[:].rearrange("p (c e) -> p c e", e=P),
                                in_ap=src_aps[stream],
                                idxs_ap=idx_tiles[stream][:, t * 64:(t + 1) * 64],
                                num_idxs=NG * P, num_idxs_reg=NG * P, elem_size=P)
                            cache[key] = gt
                        return cache[key][:].rearrange("p (c e) -> p c e", e=P)

                    # Q-tiled BN constants for the batched epilogue
                    Q = 7
                    bnq = mk.tile([P, 3 * Q * F], F32, tag="bnq", bufs=1,
                                  name="bnq")
                    for q in range(Q):
                        nc.vector.tensor_copy(out=bnq[:, q * F:(q + 1) * F], in_=gb)
                        nc.vector.tensor_copy(
                            out=bnq[:, (Q + q) * F:(Q + q + 1) * F], in_=ss)
                        nc.vector.tensor_copy(
                            out=bnq[:, (2 * Q + q) * F:(2 * Q + q + 1) * F], in_=ts)

                    nslot = BPC // 2 if probe == "half" else BPC
                    stg = None
                    batch_s0 = 0
                    for s in range(nslot):
                        if s % Q == 0:
                            stg = sm.tile([P, Q * 68], F32, tag="stg", bufs=2,
                                          name="stg")
                            batch_s0 = s
                        acc = psa.tile([P, 68], F32, tag="acc", name="acc")
                        for b in range(NB):
                            g3s = group("l" if b < CL // 8 else "h",
                                        s * (CL // 8) + b if b < CL // 8
                                        else s * (CH // 8) + (b - CL // 8))
                            g3d = group("d", s * NB + b)
                            c0 = s * NCH + b * 8
                            S8 = mk.tile([P, 8 * P], BF16, tag="S8", name="S8")
                            nc.vector.tensor_tensor(
                                out=S8[:].rearrange("p (c e) -> p c e", e=P),
                                in0=iotar[:].rearrange("p (c e) -> p c e", e=P),
                                in1=dlt[:, c0:c0 + 8].to_broadcast([P, 8, P]),
                                op=mybir.AluOpType.is_equal)
                            e1 = sm.tile([P, 32], BF16, tag="e1", name="e1")
                            nc.vector.tensor_tensor(
                                out=e1[:].rearrange("p (c e) -> p c e", e=4),
                                in0=g3s[:, :, 64:68], in1=g3d[:, :, 68:72],
                                op=mybir.AluOpType.add)
                            ex = sm.tile([P, 32], BF16, tag="ex", name="ex")
                            nc.scalar.activation(out=ex[:], in_=e1[:], func=A.Lrelu,
                                                 alpha=0.2)
                            msg = sm.tile([P, 8 * 68], BF16, tag="msg", name="msg")
                            msg3 = msg[:].rearrange("p (c e) -> p c e", e=68)
                            nc.scalar.activation(out=msg3[:, :, 64:68],
                                                 in_=ex[:].rearrange(
                                                     "p (c e) -> p c e", e=4),
                                                 func=A.Exp)
                            nc.vector.tensor_tensor(
                                out=msg3[:, :, 0:64], in0=g3s[:, :, 0:64],
                                in1=msg3[:, :, 64:68].to_broadcast([P, 8, 4, 16]),
                                op=mybir.AluOpType.mult)
                            for k in range(8):
                                j = b * 8 + k
                                if j in dead:
                                    continue
                                nc.tensor.matmul(
                                    out=acc[:], lhsT=S8[:, k * P:(k + 1) * P],
                                    rhs=msg[:, k * 68:(k + 1) * 68],
                                    start=(j == 0), stop=(j == CL + CHr - 1))
                        qi = s - batch_s0
                        # stage to SBUF; +1e-16 keeps pad-row den > 0 and is
                        # a no-op on real feature/den magnitudes
                        nc.scalar.activation(out=stg[:, qi * 68:(qi + 1) * 68],
                                             in_=acc[:], func=A.Copy, bias=1e-16)
                        if qi == Q - 1 or s == nslot - 1:
                            nq = qi + 1
                            stg3 = stg[:].rearrange("p (c e) -> p c e", e=68)
                            rdq = epp.tile([P, Q * 4], F32, tag="rdq", name="rdq")
                            nc.vector.reciprocal(
                                rdq[:, :nq * 4].rearrange("p (c e) -> p c e", e=4),
                                stg3[:, 0:nq, 64:68])
                            hgq = epp.tile([P, Q * F], F32, tag="hgq", name="hgq")
                            nc.vector.tensor_tensor(
                                out=hgq[:, :nq * F].rearrange("p (c e) -> p c e",
                                                              e=F),
                                in0=stg3[:, 0:nq, 0:64],
                                in1=rdq[:, :nq * 4].to_broadcast([P, nq * 4, 16]),
                                op=mybir.AluOpType.mult)
                            nc.vector.tensor_tensor(out=hgq[:, :nq * F],
                                                    in0=hgq[:, :nq * F],
                                                    in1=bnq[:, :nq * F],
                                                    op=mybir.AluOpType.add)
                            nc.vector.tensor_scalar_max(hgq[:, :nq * F],
                                                        hgq[:, :nq * F], 0.0)
                            nc.vector.tensor_tensor(
                                out=hgq[:, :nq * F], in0=hgq[:, :nq * F],
                                in1=bnq[:, Q * F:Q * F + nq * F],
                                op=mybir.AluOpType.mult)
                            nc.vector.tensor_tensor(
                                out=hgq[:, :nq * F], in0=hgq[:, :nq * F],
                                in1=bnq[:, 2 * Q * F:2 * Q * F + nq * F],
                                op=mybir.AluOpType.add)
                            for q in range(nq):
                                epilogue(batch_s0 + q,
                                         hgq[:, q * F:(q + 1) * F],
                                         mk, sm, epp, pst, pso)

            # ---- layer-1 aggregation, fused with layer-2 stage A ----
            def epi1(s, hg, mk, sm, epp, pst, pso):
                hgT_p = pst.tile([F, P], F32, tag="tp")
                nc.tensor.transpose(out=hgT_p[:], in_=hg, identity=ident[:])
                hgT = epp.tile([F, P], BF16, tag="hgT")
                nc.scalar.activation(out=hgT[:], in_=hgT_p[:], func=A.Copy)
                ps2 = pso.tile([P, 72], F32, tag="ps2")
                nc.tensor.matmul(out=ps2[:], lhsT=hgT[:], rhs=wct2[:],
                                 start=True, stop=True)
                st2 = sm.tile([P, P], BF16, tag="st2")
                nc.scalar.activation(out=st2[:, :72], in_=ps2[:], func=A.Copy)
                nc.vector.memset(st2[:, 72:], 0.0)
                nc.sync.dma_start(sa2_in[P * s:P * (s + 1), :], st2[:])

            aggregate(sa1, sa1_in, gbt1, sst1, tst1, epi1)

            nc.gpsimd.collective_compute(
                "AllGather", mybir.AluOpType.bypass, replica_groups=RG,
                ins=[sa2_in[:].opt()], outs=[sa2[:].opt()])

            # ---- layer-2 aggregation, fused with sum-pool partials ----
            pooled_holder = {}

            SLAST = (BPC // 2 if probe == "half" else BPC) - 1

            def epi2(s, hg, mk, sm, epp, pst, pso):
                if "ps" not in pooled_holder:
                    pooled_holder["ps"] = pso.tile([F, 512], F32, tag="pool",
                                                   bufs=1, name="pooled_ps")
                pm = mk.tile([P, 512], F32, tag="pm")
                nc.vector.tensor_scalar(
                    out=pm[:], in0=iota5[:], scalar1=blt[:, s:s + 1],
                    scalar2=None, op0=mybir.AluOpType.is_equal)
                nc.tensor.matmul(out=pooled_holder["ps"][:], lhsT=hg, rhs=pm[:],
                                 start=(s == 0), stop=(s == SLAST))
                if s == SLAST:
                    po = epp.tile([F, 512], BF16, tag="po")
                    nc.scalar.activation(out=po[:], in_=pooled_holder["ps"][:],
                                         func=A.Copy)
                    nc.sync.dma_start(pooledT[:], po[:])

            aggregate(sa2, sa2_in, gbt2, sst2, tst2, epi2)
    nc.compile()
    # The PJRT lowering re-serializes the BIR module (to_json_bytes) on
    # every launch; the module is frozen after compile, so memoize it.
    _json = nc.to_json_bytes()
    nc.to_json_bytes = lambda: _json
    return nc


def _fold_bn(g, b, m, v):
    s = np.asarray(g) / np.sqrt(np.asarray(v) + BN_EPS)
    return s.astype(np.float32), (np.asarray(b) - np.asarray(m) * s).astype(np.float32)


def _layer_consts(W, bias, asrc, adst, bn_g, bn_b, bn_m, bn_v):
    W = np.asarray(W, np.float32)
    As = np.zeros((F, H), np.float32)
    Ad = np.zeros((F, H), np.float32)
    for hd in range(H):
        As[hd * CH_:(hd + 1) * CH_, hd] = np.asarray(asrc)[hd]
        Ad[hd * CH_:(hd + 1) * CH_, hd] = np.asarray(adst)[hd]
    wcm = np.concatenate([W, W @ As, W @ Ad], axis=1).astype(np.float32)
    s, t = _fold_bn(bn_g, bn_b, bn_m, bn_v)
    cst = np.stack([
        np.tile(np.asarray(bias, np.float32)[None, :], (P, 1)),
        np.tile(s[None, :], (P, 1)),
        np.tile(t[None, :], (P, 1)),
    ]).astype(np.float32)
    return wcm, cst


def _sigmoid(x):
    return 1.0 / (1.0 + np.exp(-x))


def _bn_np(x, g, b, m, v):
    return (x - m) / np.sqrt(v + BN_EPS) * g + b


def _heads(inp, pooled):
    f = lambda k: np.asarray(inp[k], np.float32)
    ya = np.maximum(pooled @ f("la1_w") + f("la1_b"), 0.0)
    xa = _sigmoid(ya @ f("la2_w") + f("la2_b"))            # [G, 1]
    z = f("x2")
    for i in (1, 2, 3):
        z = np.maximum(_bn_np(z @ f(f"lb{i}_w") + f(f"lb{i}_b"),
                              f(f"bnb{i}_g"), f(f"bnb{i}_b"),
                              f(f"bnb{i}_m"), f(f"bnb{i}_v")), 0.0)
    xb = _sigmoid(z @ f("lb4_w") + f("lb4_b"))             # [G, 64]
    c = np.concatenate([xa, xb], axis=1)                   # [G, 65]
    yc = np.maximum(c @ f("lc1_w") + f("lc1_b"), 0.0)
    return _sigmoid(yc @ f("lc2_w") + f("lc2_b")).astype(np.float32)


_CACHE = {}
LAUNCH_S = []      # all launches ever (name, wall seconds)
LAST_CALL = []     # launches of the most recent kernel() call


def kernel(**inputs):
    global LAST_CALL
    edge_index = inputs["edge_index"]
    batch = np.asarray(inputs["batch"]).astype(np.int64)
    CL, CH, CLr, CHr, idxL, idxH, idxD, dl, bl = _prep_graph(edge_index, batch)

    key = (CL, CH, CLr, CHr)
    if key not in _CACHE:
        _CACHE[key] = _build_fused(CL, CH, CLr, CHr)
    nc = _CACHE[key]

    w1c, cst1 = _layer_consts(inputs["gW1"], inputs["gb1"], inputs["asrc1"],
                              inputs["adst1"], inputs["bn1_g"], inputs["bn1_b"],
                              inputs["bn1_m"], inputs["bn1_v"])
    w2c, cst2 = _layer_consts(inputs["gW2"], inputs["gb2"], inputs["asrc2"],
                              inputs["adst2"], inputs["bn2_g"], inputs["bn2_b"],
                              inputs["bn2_m"], inputs["bn2_v"])
    import ml_dtypes
    x1T = np.zeros((F, NPAD), ml_dtypes.float8_e4m3)
    x1T[:, :N] = np.asarray(inputs["x1"], np.float32).T.astype(ml_dtypes.float8_e4m3)

    def pack(c):
        parts = [np.ascontiguousarray(x1T[:, c * NPC:(c + 1) * NPC]),
                 idxL[c], idxH[c], idxD[c], dl[c], bl[c],
                 w1c.astype(ml_dtypes.bfloat16), w2c.astype(ml_dtypes.bfloat16),
                 cst1.astype(ml_dtypes.bfloat16), cst2.astype(ml_dtypes.bfloat16)]
        return np.concatenate([p.reshape(-1).view(np.int8) for p in parts])

    maps = [{"blob": pack(c)} for c in range(NCORE)]
    t0 = time.time()
    res = run_bass_kernel_spmd(nc, maps, core_ids=list(range(NCORE)))
    dt = time.time() - t0
    LAUNCH_S.append(("FUSED", dt))
    LAST_CALL = [("FUSED", dt)]

    poolT = np.zeros((F, G), np.float32)
    for c in range(NCORE):
        poolT += res.results[c]["pooledT"].astype(np.float32)
    cnt = np.bincount(batch, minlength=G).astype(np.float32)
    pooled = (poolT / np.maximum(cnt, 1.0)[None, :]).T     # [G, F]
    return _heads(inputs, pooled)
